# revision 53
# baseline (speedup 1.0000x reference)
"""Trainium2 Bass kernel: MultiHeadLatentAttention prefill (B=2, T=2048, D=2048,
H=16, HD=128, KVH=4, QL=1536, KVL=512).

Sharding: 8 cores = (batch b in {0,1}) x (kv-head group g in {0..3}).
Each core handles one batch element and the 4 q-heads of one kv group.

v2: the low-rank two-step projections are folded on the host (the rms scale
is a per-token SCALAR, so Wuq@rms(Wdq@x) == sq[t] * (Wuq@Wdq)@x):
  Wq_eff  = (Wuq *qw)@Wdq   per group [512, D]
  Wqr_eff = (Wqr *qw)@Wdq   per group [512, D]
  Wk_eff  = (Wuk*kvw)@Wdkv  shared    [128, D]   (absorbed-Wuk key)
  Wv_eff  = (Wuv*kvw)@Wdkv  per group [128, D]
  Wkr_eff = rope-folded Wkr per group [128, D]
The rms denominators need the *unfolded* ql/c rows: each core computes a
1/4 slice (Wdq 384 rows, Wdkv 128 rows) in fp8 DoubleRow (2x PE rate; norms
average the quantization noise away), and the [1,2,T] sum-of-squares is
AllReduced across the 4 cores of each batch (groups [[0-3],[4-7]]) while
the main projection matmuls keep the PE busy.

Device dataflow (feature-major activations [feat, T], bf16 matmuls, fp32
PSUM; per-token scales applied at consumer epilogues):
  norm slices (fp8 DR) -> squares (Act) -> ones-matmul -> AllReduce -> sq,sc
  K_T    = Wkr_eff@x            (rope folded on host)
  Kabs_T = (Wk_eff@x) * sc[t]
  V^T    = (x_slices^T@Wv_eff^T) * sc[s]    (token-major directly)
  Q_T    = (Wq_eff@x) * sq[t]
  Qr_T   = rope(Wqr_eff@x) with tables pre-scaled by sq[t]
  per head: S_T[s,t] = Kabs^T@Q + K^T@Qr ; E = exp(S/16)*mask
            Z = ones^T@E ; ctx = (V^T)^T@E / Z
  out_T  = Wout_g^T @ ctx  (partial over groups; host sums)
"""

import numpy as np
import ml_dtypes

B, T, D = 2, 2048, 2048
H, HD, KVH = 16, 128, 4
QL, KVL = 1536, 512
G = KVH
HPG = H // KVH           # 4 q heads per group
NCORES = B * G
TS = 512
NT = T // TS             # 4
DCH = D // 128           # 16
SCH = T // 128           # 16
QSL = QL // G            # 384 q-norm rows per core
CSL = KVL // G           # 128 c-norm rows per core
NF = HPG * HD * 2 + HD * 3   # 1408 folded-projection features
NFCH = NF // 128             # 11
EPS = 1e-6
SM_SCALE = 1.0 / 16.0
NSCALE = 32.0            # norm-slice fp8 weight pre-scale
BF16 = ml_dtypes.bfloat16
FP8 = ml_dtypes.float8_e4m3

SCORES_FP8 = False       # quantize Q/Qr/Kabs/K to fp8, DoubleRow scores
Z_FP8 = False            # fp8 copy of E for DoubleRow softmax-denominator

_CACHE = {}
LAST_RESULTS = None


def _build_program(reps=1, phases=3, no_cc=False, att_mode='full', fuse_out=True):
    import concourse.bacc as bacc
    import concourse.tile as tile
    from concourse import mybir
    from concourse.bass import ts

    bf = mybir.dt.bfloat16
    f32 = mybir.dt.float32
    fp8 = mybir.dt.float8e4
    AF = mybir.ActivationFunctionType
    DR = mybir.MatmulPerfMode.DoubleRow
    SWAP_MASK = [i ^ 1 for i in range(32)]

    nc = bacc.Bacc("TRN2", target_bir_lowering=False, debug=False)

    xT = nc.dram_tensor("x_T", [D, T], bf, kind="ExternalInput")
    x8d = nc.dram_tensor("x8", [D, T], fp8, kind="ExternalInput")
    wstT = nc.dram_tensor("wstT", [D, NF], bf, kind="ExternalInput")
    wn8T = nc.dram_tensor("wn8T", [D, QL + KVL], fp8, kind="ExternalInput")
    woutT = nc.dram_tensor("woutT", [HPG * HD, D], bf, kind="ExternalInput")
    ctab = nc.dram_tensor("ctab", [HD, T], bf, kind="ExternalInput")
    stab = nc.dram_tensor("stab", [HD, T], bf, kind="ExternalInput")
    masksd = nc.dram_tensor("masks", [128, 4, TS], bf, kind="ExternalInput")
    outT = nc.dram_tensor("out_T", [D, T], f32, kind="ExternalOutput")

    # feature-chunk schedule for the main projection: K first (its epilogue
    # needs no norm scale, buying time for the collective), then Kabs, V,
    # Q, Qr
    CH_K, CH_KABS, CH_V = 9, 8, 10
    CH_ORDER = [CH_K, CH_KABS, CH_V, 0, 1, 2, 3, 4, 5, 6, 7]

    with tile.TileContext(nc) as tc:
      for _rep in range(reps):
        with tc.tile_pool(name="A", bufs=1) as A:
            if SCORES_FP8:
                q_sb = A.tile([128, HPG, 2, T], fp8)     # [.,h,0,.]=Q [.,h,1,.]=Qr
                keys_sb = A.tile([128, 2, T], fp8)       # [.,0,.]=Kabs [.,1,.]=K
            else:
                q_sb = A.tile([128, HPG, T], bf)
                qr_sb = A.tile([128, HPG, T], bf)
                kabs_sb = A.tile([128, T], bf)
                k_sb = A.tile([128, T], bf)
            v_sb = A.tile([128, SCH, HD], bf)
            onesm_sb = A.tile([128, 128], bf)
            sq_bc = A.tile([128, T], bf)
            sc_bc = A.tile([128, T], bf)
            sc_col = A.tile([128, SCH], f32)
            sc_colb = A.tile([128, SCH], bf)
            eps_sb = A.tile([128, 1], f32)
            ctab_sb = A.tile([128, T], bf)  # becomes ctab*sq in place
            stab_sb = A.tile([128, T], bf)
            if Z_FP8:
                ones8_sb = A.tile([128, 2, 128], fp8)

            # ---------------- phase 1: projections ---------------------
            with (
                tc.tile_pool(name="PH1", bufs=1) as P1,
                tc.tile_pool(name="P1S", bufs=2) as P1S,
                tc.tile_pool(name="SQS", bufs=2) as SQS,
                tc.tile_pool(name="DRS", bufs=1, space="DRAM") as DRS,
                tc.tile_pool(name="X8P", bufs=2) as X8P,
                tc.tile_pool(name="PP1", bufs=6, space="PSUM") as PP1,
                tc.tile_pool(name="PZ1", bufs=2, space="PSUM") as PZ1,
            ):
                wn8_sb = P1.tile([128, DCH, QL + KVL], fp8)
                wn8_r = wn8T.ap().rearrange("(c p) f -> p c f", p=128)
                for dp in range(DCH // 2):
                    nc.sync.dma_start(
                        out=wn8_sb[:, 2 * dp:2 * dp + 2, :],
                        in_=wn8_r[:, 2 * dp:2 * dp + 2, :],
                    )
                x8_ts = []
                for t in range(NT):
                    x8_t = X8P.tile([128, DCH, TS], fp8, tag="x8")
                    nc.sync.dma_start(
                        out=x8_t,
                        in_=x8d.ap()[:, t * TS:(t + 1) * TS].rearrange(
                            "(c p) t -> p c t", p=128
                        ),
                    )
                    x8_ts.append(x8_t)
                nc.vector.memset(onesm_sb, 1.0)
                nc.vector.memset(eps_sb, EPS)
                if Z_FP8:
                    nc.vector.tensor_copy(ones8_sb[:, 0, :], onesm_sb)
                    nc.vector.tensor_copy(ones8_sb[:, 1, :], onesm_sb)
                xT_r = xT.ap().rearrange("(c p) t -> c p t", p=128)
                x_sb = []
                for d in range(DCH):
                    xd = P1.tile([128, T], bf, tag=f"x{d}", name=f"x{d}")
                    nc.gpsimd.dma_start(out=xd, in_=xT_r[d])
                    x_sb.append(xd)
                nc.sync.dma_start(out=ctab_sb, in_=ctab.ap())
                nc.sync.dma_start(out=stab_sb, in_=stab.ap())

                w_ts = []
                for m in range(NFCH):
                    w_t = P1S.tile([128, DCH, 128], bf, tag="wstream")
                    f0 = CH_ORDER[m] * 128
                    nc.sync.dma_start(
                        out=w_t,
                        in_=wstT.ap()[:, f0:f0 + 128].rearrange(
                            "(c p) f -> p c f", p=128
                        ),
                    )
                    w_ts.append(w_t)

                # --- full norms (fp8 DoubleRow) + sum-of-squares ---
                # ones_mat reduction puts the sum in EVERY psum partition, so
                # sqrt/reciprocal write the [128,T] scale broadcast directly
                NQCH = QL // 128      # 12 qn chunks
                NCCH_T = (QL + KVL) // 128   # 16 total
                for t in range(NT):
                    psz_q = PZ1.tile([128, TS], f32, tag="pz", name="pszq")
                    psz_c = PZ1.tile([128, TS], f32, tag="pz", name="pszc")
                    for c in range(NCCH_T):
                        is_c = c >= NQCH
                        ps = PP1.tile([128, TS], f32, tag="pp")
                        for dp in range(DCH // 2):
                            nc.tensor.matmul(
                                ps,
                                lhsT=wn8_sb[:, 2 * dp:2 * dp + 2,
                                            c * 128:(c + 1) * 128],
                                rhs=x8_ts[t][:, 2 * dp:2 * dp + 2, :],
                                start=(dp == 0),
                                stop=(dp == DCH // 2 - 1),
                                perf_mode=DR,
                            )
                        sqv = SQS.tile([128, TS], bf, tag="sqs")
                        nc.scalar.activation(sqv, ps, AF.Square)
                        psz = psz_c if is_c else psz_q
                        nc.tensor.matmul(
                            psz,
                            lhsT=onesm_sb,
                            rhs=sqv,
                            start=(c == 0 or c == NQCH),
                            stop=(c == NQCH - 1 or c == NCCH_T - 1),
                        )
                        if c == NQCH - 1:
                            nc.scalar.activation(
                                sq_bc[:, ts(t, TS)], psz_q, AF.Sqrt,
                                bias=eps_sb,
                                scale=1.0 / (QL * NSCALE * NSCALE),
                            )
                        if c == NCCH_T - 1:
                            nc.scalar.activation(
                                sc_bc[:, ts(t, TS)], psz_c, AF.Sqrt,
                                bias=eps_sb,
                                scale=1.0 / (KVL * NSCALE * NSCALE),
                            )
                with nc.allow_low_precision(reason="rms scales bf16"):
                    nc.vector.reciprocal(sq_bc, sq_bc)
                    nc.vector.reciprocal(sc_bc, sc_bc)
                dr = DRS.tile([1, T], bf)
                nc.sync.dma_start(out=dr, in_=sc_bc[0:1, :])
                nc.sync.dma_start(
                    out=sc_colb,
                    in_=dr[:, :].rearrange("o (s p) -> (o p) s", p=128),
                )
                nc.vector.tensor_copy(sc_col, sc_colb)
                # rope tables pre-scaled by sq (in place)
                nc.vector.tensor_mul(ctab_sb, ctab_sb, sq_bc)
                nc.vector.tensor_mul(stab_sb, stab_sb, sq_bc)

                # --- main projection chunks ---
                for m in range(NFCH):
                    ch = CH_ORDER[m]
                    w_t = w_ts[m]
                    if ch == CH_V:
                        # V^T token-major: x slices are the stationary side
                        for s in range(SCH):
                            ps = PP1.tile([128, TS], f32, tag="pp")
                            for d in range(DCH):
                                nc.tensor.matmul(
                                    ps[:, 0:HD],
                                    lhsT=x_sb[d][:, s * 128:(s + 1) * 128],
                                    rhs=w_t[:, d, :],
                                    start=(d == 0),
                                    stop=(d == DCH - 1),
                                )
                            nc.vector.tensor_scalar_mul(
                                v_sb[:, s, :], ps[:, 0:HD], sc_col[:, s:s + 1]
                            )
                        continue
                    for t in range(NT):
                        ps = PP1.tile([128, TS], f32, tag="pp")
                        for d in range(DCH):
                            nc.tensor.matmul(
                                ps,
                                lhsT=w_t[:, d, :],
                                rhs=x_sb[d][:, ts(t, TS)],
                                start=(d == 0),
                                stop=(d == DCH - 1),
                            )
                        if ch == CH_K:
                            dst = (keys_sb[:, 1, ts(t, TS)] if SCORES_FP8
                                   else k_sb[:, ts(t, TS)])
                            nc.vector.tensor_copy(dst, ps)
                        elif ch == CH_KABS:
                            dst = (keys_sb[:, 0, ts(t, TS)] if SCORES_FP8
                                   else kabs_sb[:, ts(t, TS)])
                            nc.vector.tensor_mul(dst, ps, sc_bc[:, ts(t, TS)])
                        elif ch < HPG:
                            h = ch
                            dst = (q_sb[:, h, 0, ts(t, TS)] if SCORES_FP8
                                   else q_sb[:, h, ts(t, TS)])
                            nc.vector.tensor_mul(dst, ps, sq_bc[:, ts(t, TS)])
                        else:
                            h = ch - HPG
                            tsw = SQS.tile([128, TS], f32, tag="ropef")
                            nc.vector.stream_shuffle(tsw, ps, SWAP_MASK)
                            t1 = SQS.tile([128, TS], bf, tag="rope")
                            nc.vector.tensor_mul(t1, ps, ctab_sb[:, ts(t, TS)])
                            t2 = SQS.tile([128, TS], bf, tag="rope")
                            nc.vector.tensor_mul(t2, tsw, stab_sb[:, ts(t, TS)])
                            dst = (q_sb[:, h, 1, ts(t, TS)] if SCORES_FP8
                                   else qr_sb[:, h, ts(t, TS)])
                            nc.vector.tensor_add(dst, t1, t2)

            # ---------------- phases 3+4: attention + out proj ---------
            if phases < 2:
                continue
            with tc.tile_pool(name="P3B", bufs=1) as P3B:
                ctx_sb = [
                    [
                        P3B.tile([128, TS], bf, tag=f"ctx{h}_{j}",
                                 name=f"ctx{h}_{j}")
                        for j in range(NT)
                    ]
                    for h in range(HPG)
                ]
                wout_sb = P3B.tile([128, HPG, T], bf)
                nc.gpsimd.dma_start(
                    out=wout_sb,
                    in_=woutT.ap().rearrange("(c p) e -> p c e", p=128),
                )
                masks_sb = P3B.tile([128, 4, TS], bf)
                nc.sync.dma_start(out=masks_sb, in_=masksd.ap())
                with (
                    tc.tile_pool(name="EP", bufs=8) as EP,
                    tc.tile_pool(name="TMPP", bufs=4) as TMPP,
                    tc.tile_pool(name="ZR", bufs=4) as ZR,
                    tc.tile_pool(name="OT", bufs=3) as OT,
                    tc.tile_pool(name="PSC", bufs=2, space="PSUM") as PSC,
                    tc.tile_pool(name="PCT", bufs=2, space="PSUM") as PCT,
                    tc.tile_pool(name="PZ3", bufs=2, space="PSUM") as PZ3,
                    tc.tile_pool(name="PO", bufs=2, space="PSUM") as PO,
                ):
                    # s-loop software pipeline: Z/ctx matmuls run LAG blocks
                    # behind the scores so the PE never waits on the
                    # PE->Act->DVE->PE e_t round-trip. The out-projection for
                    # query tile j-1 is emitted after attention tile j, so its
                    # ctx dependency is long satisfied when the PE reaches it.
                    LAG = 3

                    def emit_wout(jj):
                        for e in range(DCH):
                            po = PO.tile([128, TS], f32, tag="po")
                            for q in range(HPG):
                                nc.tensor.matmul(
                                    po,
                                    lhsT=wout_sb[:, q, e * 128:(e + 1) * 128],
                                    rhs=ctx_sb[q][jj],
                                    start=(q == 0),
                                    stop=(q == HPG - 1),
                                )
                            o_t = OT.tile([128, TS], f32, tag="ot")
                            nc.scalar.copy(o_t, po)
                            nc.gpsimd.dma_start(
                                out=outT.ap()[e * 128:(e + 1) * 128,
                                              jj * TS:(jj + 1) * TS],
                                in_=o_t,
                            )

                    for j in range(NT):
                        n_s = 4 * (j + 1)
                        for h in range(HPG):
                            psz = PZ3.tile([128, TS], f32, tag="pz3",
                                           name="psz")
                            pctx = PCT.tile([128, TS], f32, tag="pct",
                                            name="pctx")
                            e_list = {}
                            e8 = {}
                            for sv in range(n_s + LAG):
                                if sv < n_s:
                                    s_i = sv
                                    pss = PSC.tile([128, TS], f32, tag="psc")
                                    if SCORES_FP8:
                                        nc.tensor.matmul(
                                            pss,
                                            lhsT=keys_sb[:, :,
                                                         s_i * 128:(s_i + 1) * 128],
                                            rhs=q_sb[:, h, :, ts(j, TS)],
                                            start=True,
                                            stop=True,
                                            perf_mode=DR,
                                        )
                                    else:
                                        nc.tensor.matmul(
                                            pss,
                                            lhsT=kabs_sb[:,
                                                         s_i * 128:(s_i + 1) * 128],
                                            rhs=q_sb[:, h, ts(j, TS)],
                                            start=True,
                                            stop=False,
                                        )
                                        nc.tensor.matmul(
                                            pss,
                                            lhsT=k_sb[:, s_i * 128:(s_i + 1) * 128],
                                            rhs=qr_sb[:, h, ts(j, TS)],
                                            start=False,
                                            stop=True,
                                        )
                                    e_t = EP.tile([128, TS], bf, tag="e")
                                    nc.scalar.activation(e_t, pss, AF.Exp,
                                                         scale=SM_SCALE)
                                    if s_i >= 4 * j:
                                        nc.vector.tensor_mul(
                                            e_t, e_t, masks_sb[:, s_i - 4 * j, :]
                                        )
                                    e_list[s_i] = e_t
                                    if Z_FP8:
                                        if s_i % 2 == 0:
                                            e8[s_i // 2] = EP.tile(
                                                [128, 2, TS], fp8, tag="e8",
                                                name="e8t")
                                        nc.vector.tensor_copy(
                                            e8[s_i // 2][:, s_i % 2, :], e_t
                                        )
                                if sv >= LAG:
                                    s_o = sv - LAG
                                    et = e_list.pop(s_o)
                                    if Z_FP8:
                                        if s_o % 2 == 1:
                                            nc.tensor.matmul(
                                                psz,
                                                lhsT=ones8_sb,
                                                rhs=e8.pop(s_o // 2),
                                                start=(s_o == 1),
                                                stop=(s_o == n_s - 1),
                                                perf_mode=DR,
                                            )
                                    else:
                                        nc.tensor.matmul(
                                            psz,
                                            lhsT=onesm_sb,
                                            rhs=et,
                                            start=(s_o == 0),
                                            stop=(s_o == n_s - 1),
                                        )
                                    nc.tensor.matmul(
                                        pctx,
                                        lhsT=v_sb[:, s_o, :],
                                        rhs=et,
                                        start=(s_o == 0),
                                        stop=(s_o == n_s - 1),
                                    )
                            zcp = ZR.tile([128, TS], f32, tag="zcp")
                            nc.scalar.copy(zcp, psz)
                            zinv = ZR.tile([128, TS], f32, tag="zrow")
                            nc.vector.reciprocal(zinv, zcp)
                            nc.vector.tensor_mul(ctx_sb[h][j], pctx, zinv)
                        if fuse_out and j > 0:
                            emit_wout(j - 1)
                    if fuse_out:
                        emit_wout(NT - 1)
                    else:
                        for jj in range(NT):
                            emit_wout(jj)

    nc.compile()
    return nc


def _get_program():
    if "nc" not in _CACHE:
        _CACHE["nc"] = _build_program()
    return _CACHE["nc"]


def _host_prep(inputs):
    """Fold weights on the host and build the 8 per-core input maps."""
    x = np.asarray(inputs["x"], np.float32)
    Wdq = np.asarray(inputs["Wdq"], np.float32)
    qw = np.asarray(inputs["q_norm_w"], np.float32)
    Wuq = np.asarray(inputs["Wuq"], np.float32) * qw[None, :]
    Wqr = np.asarray(inputs["Wqr"], np.float32) * qw[None, :]
    Wdkv = np.asarray(inputs["Wdkv"], np.float32)
    kvw = np.asarray(inputs["kv_norm_w"], np.float32)
    Wuk = np.asarray(inputs["Wuk"], np.float32) * kvw[None, :]
    Wuv = np.asarray(inputs["Wuv"], np.float32) * kvw[None, :]
    Wkr = np.asarray(inputs["Wkr"], np.float32)
    Wout = np.asarray(inputs["Wout"], np.float32)

    # folded projections
    Wq_eff = Wuq @ Wdq          # [H*HD, D]
    Wqr_eff = Wqr @ Wdq
    Wk_eff = Wuk @ Wdkv         # [HD, D]
    Wv_eff = Wuv @ Wdkv         # [KVH*HD, D]

    inv = 1.0 / (10000.0 ** (np.arange(0, HD, 2, dtype=np.float32) / HD))
    f = np.arange(T, dtype=np.float32)[None, :] * inv[:, None]   # [64, T]
    cosT, sinT = np.cos(f), np.sin(f)
    Ctab = np.repeat(cosT, 2, axis=0)                            # [128, T]
    Stab = np.repeat(sinT, 2, axis=0)
    Stab[0::2, :] *= -1.0                                        # pair-swap sign

    fH = np.arange(KVH, dtype=np.float32)[None, :] * inv[:, None]  # [64, KVH]
    cosH, sinH = np.cos(fH), np.sin(fH)

    def bft(a):
        return np.ascontiguousarray(a).astype(BF16)

    def f8t(a):
        return np.ascontiguousarray(a).astype(FP8)

    ctab_b = bft(Ctab)
    stab_b = bft(Stab)
    masks_np = np.ones((128, 4, TS), np.float32)
    for r in range(4):
        for p in range(128):
            masks_np[p, r, :p + 128 * r] = 0.0
    masks_b = bft(masks_np)

    in_maps = []
    for b in range(B):
        x_T = bft(x[b].T)
        x8 = f8t(x[b].T)
        for g in range(G):
            # fold K-rope (fixed rotation per kv-head index) into Wkr
            Wkr_g = Wkr[g * HD:(g + 1) * HD, :]
            we, wo = Wkr_g[0::2, :], Wkr_g[1::2, :]
            c_g, s_g = cosH[:, g][:, None], sinH[:, g][:, None]
            Wkr_eff = np.empty_like(Wkr_g)
            Wkr_eff[0::2, :] = we * c_g - wo * s_g
            Wkr_eff[1::2, :] = we * s_g + wo * c_g

            wst = np.concatenate(
                [
                    Wq_eff[g * HPG * HD:(g + 1) * HPG * HD],
                    Wqr_eff[g * HPG * HD:(g + 1) * HPG * HD],
                    Wk_eff,
                    Wkr_eff,
                    Wv_eff[g * HD:(g + 1) * HD],
                ],
                axis=0,
            )  # [NF, D]
            wn = np.concatenate([Wdq, Wdkv], axis=0) * NSCALE  # [QL+KVL, D]

            in_maps.append(
                dict(
                    x_T=x_T,
                    x8=x8,
                    wstT=bft(wst.T),
                    wn8T=f8t(wn.T),
                    woutT=bft(Wout[:, g * HPG * HD:(g + 1) * HPG * HD].T),
                    ctab=ctab_b,
                    stab=stab_b,
                    masks=masks_b,
                )
            )
    return in_maps


def kernel(**inputs):
    global LAST_RESULTS
    from concourse import bass_utils

    nc = _get_program()
    in_maps = _host_prep(inputs)
    res = bass_utils.run_bass_kernel_spmd(
        nc, in_maps, core_ids=list(range(NCORES))
    )
    LAST_RESULTS = res
    out = np.zeros((B, T, D), np.float32)
    for i, r in enumerate(res.results):
        out[i // G] += r["out_T"].T
    return out


# revision 54
# speedup vs baseline: 1.0387x; 1.0387x over previous
"""Trainium2 Bass kernel: MultiHeadLatentAttention prefill (B=2, T=2048, D=2048,
H=16, HD=128, KVH=4, QL=1536, KVL=512).

Sharding: 8 cores = (batch b in {0,1}) x (kv-head group g in {0..3}).
Each core handles one batch element and the 4 q-heads of one kv group.

v2: the low-rank two-step projections are folded on the host (the rms scale
is a per-token SCALAR, so Wuq@rms(Wdq@x) == sq[t] * (Wuq@Wdq)@x):
  Wq_eff  = (Wuq *qw)@Wdq   per group [512, D]
  Wqr_eff = (Wqr *qw)@Wdq   per group [512, D]
  Wk_eff  = (Wuk*kvw)@Wdkv  shared    [128, D]   (absorbed-Wuk key)
  Wv_eff  = (Wuv*kvw)@Wdkv  per group [128, D]
  Wkr_eff = rope-folded Wkr per group [128, D]
The rms denominators need the *unfolded* ql/c rows: each core computes a
1/4 slice (Wdq 384 rows, Wdkv 128 rows) in fp8 DoubleRow (2x PE rate; norms
average the quantization noise away), and the [1,2,T] sum-of-squares is
AllReduced across the 4 cores of each batch (groups [[0-3],[4-7]]) while
the main projection matmuls keep the PE busy.

Device dataflow (feature-major activations [feat, T], bf16 matmuls, fp32
PSUM; per-token scales applied at consumer epilogues):
  norm slices (fp8 DR) -> squares (Act) -> ones-matmul -> AllReduce -> sq,sc
  K_T    = Wkr_eff@x            (rope folded on host)
  Kabs_T = (Wk_eff@x) * sc[t]
  V^T    = (x_slices^T@Wv_eff^T) * sc[s]    (token-major directly)
  Q_T    = (Wq_eff@x) * sq[t]
  Qr_T   = rope(Wqr_eff@x) with tables pre-scaled by sq[t]
  per head: S_T[s,t] = Kabs^T@Q + K^T@Qr ; E = exp(S/16)*mask
            Z = ones^T@E ; ctx = (V^T)^T@E / Z
  out_T  = Wout_g^T @ ctx  (partial over groups; host sums)
"""

import numpy as np
import ml_dtypes

B, T, D = 2, 2048, 2048
H, HD, KVH = 16, 128, 4
QL, KVL = 1536, 512
G = KVH
HPG = H // KVH           # 4 q heads per group
NCORES = B * G
TS = 512
NT = T // TS             # 4
DCH = D // 128           # 16
SCH = T // 128           # 16
QSL = QL // G            # 384 q-norm rows per core
CSL = KVL // G           # 128 c-norm rows per core
NF = HPG * HD * 2 + HD * 3   # 1408 folded-projection features
NFCH = NF // 128             # 11
EPS = 1e-6
SM_SCALE = 1.0 / 16.0
NSCALE = 32.0            # norm-slice fp8 weight pre-scale
BF16 = ml_dtypes.bfloat16
FP8 = ml_dtypes.float8_e4m3

SCORES_FP8 = True       # quantize Q/Qr/Kabs/K to fp8, DoubleRow scores
Z_FP8 = False            # fp8 copy of E for DoubleRow softmax-denominator

_CACHE = {}
LAST_RESULTS = None


def _build_program(reps=1, phases=3, no_cc=False, att_mode='full', fuse_out=True):
    import concourse.bacc as bacc
    import concourse.tile as tile
    from concourse import mybir
    from concourse.bass import ts

    bf = mybir.dt.bfloat16
    f32 = mybir.dt.float32
    fp8 = mybir.dt.float8e4
    AF = mybir.ActivationFunctionType
    DR = mybir.MatmulPerfMode.DoubleRow
    SWAP_MASK = [i ^ 1 for i in range(32)]

    nc = bacc.Bacc("TRN2", target_bir_lowering=False, debug=False)

    xT = nc.dram_tensor("x_T", [D, T], bf, kind="ExternalInput")
    x8d = nc.dram_tensor("x8", [D, T], fp8, kind="ExternalInput")
    wstT = nc.dram_tensor("wstT", [D, NF], bf, kind="ExternalInput")
    wn8T = nc.dram_tensor("wn8T", [D, QL + KVL], fp8, kind="ExternalInput")
    woutT = nc.dram_tensor("woutT", [HPG * HD, D], bf, kind="ExternalInput")
    ctab = nc.dram_tensor("ctab", [HD, T], bf, kind="ExternalInput")
    stab = nc.dram_tensor("stab", [HD, T], bf, kind="ExternalInput")
    masksd = nc.dram_tensor("masks", [128, 4, TS], bf, kind="ExternalInput")
    outT = nc.dram_tensor("out_T", [D, T], f32, kind="ExternalOutput")

    # feature-chunk schedule for the main projection: K first (its epilogue
    # needs no norm scale, buying time for the collective), then Kabs, V,
    # Q, Qr
    CH_K, CH_KABS, CH_V = 9, 8, 10
    CH_ORDER = [CH_K, CH_KABS, CH_V, 0, 1, 2, 3, 4, 5, 6, 7]

    with tile.TileContext(nc) as tc:
      for _rep in range(reps):
        with tc.tile_pool(name="A", bufs=1) as A:
            if SCORES_FP8:
                q_sb = A.tile([128, HPG, 2, T], fp8)     # [.,h,0,.]=Q [.,h,1,.]=Qr
                keys_sb = A.tile([128, 2, T], fp8)       # [.,0,.]=Kabs [.,1,.]=K
            else:
                q_sb = A.tile([128, HPG, T], bf)
                qr_sb = A.tile([128, HPG, T], bf)
                kabs_sb = A.tile([128, T], bf)
                k_sb = A.tile([128, T], bf)
            v_sb = A.tile([128, SCH, HD], bf)
            onesm_sb = A.tile([128, 128], bf)
            sq_bc = A.tile([128, T], bf)
            sc_bc = A.tile([128, T], bf)
            sc_col = A.tile([128, SCH], f32)
            sc_colb = A.tile([128, SCH], bf)
            eps_sb = A.tile([128, 1], f32)
            ctab_sb = A.tile([128, T], bf)  # becomes ctab*sq in place
            stab_sb = A.tile([128, T], bf)
            if Z_FP8:
                ones8_sb = A.tile([128, 2, 128], fp8)

            # ---------------- phase 1: projections ---------------------
            with (
                tc.tile_pool(name="PH1", bufs=1) as P1,
                tc.tile_pool(name="P1S", bufs=2) as P1S,
                tc.tile_pool(name="SQS", bufs=2) as SQS,
                tc.tile_pool(name="DRS", bufs=1, space="DRAM") as DRS,
                tc.tile_pool(name="X8P", bufs=2) as X8P,
                tc.tile_pool(name="PP1", bufs=6, space="PSUM") as PP1,
                tc.tile_pool(name="PZ1", bufs=2, space="PSUM") as PZ1,
            ):
                wn8_sb = P1.tile([128, DCH, QL + KVL], fp8)
                wn8_r = wn8T.ap().rearrange("(c p) f -> p c f", p=128)
                for dp in range(DCH // 2):
                    nc.sync.dma_start(
                        out=wn8_sb[:, 2 * dp:2 * dp + 2, :],
                        in_=wn8_r[:, 2 * dp:2 * dp + 2, :],
                    )
                x8_ts = []
                for t in range(NT):
                    x8_t = X8P.tile([128, DCH, TS], fp8, tag="x8")
                    nc.sync.dma_start(
                        out=x8_t,
                        in_=x8d.ap()[:, t * TS:(t + 1) * TS].rearrange(
                            "(c p) t -> p c t", p=128
                        ),
                    )
                    x8_ts.append(x8_t)
                nc.vector.memset(onesm_sb, 1.0)
                nc.vector.memset(eps_sb, EPS)
                if Z_FP8:
                    nc.vector.tensor_copy(ones8_sb[:, 0, :], onesm_sb)
                    nc.vector.tensor_copy(ones8_sb[:, 1, :], onesm_sb)
                xT_r = xT.ap().rearrange("(c p) t -> c p t", p=128)
                x_sb = []
                for d in range(DCH):
                    xd = P1.tile([128, T], bf, tag=f"x{d}", name=f"x{d}")
                    nc.gpsimd.dma_start(out=xd, in_=xT_r[d])
                    x_sb.append(xd)
                nc.sync.dma_start(out=ctab_sb, in_=ctab.ap())
                nc.sync.dma_start(out=stab_sb, in_=stab.ap())

                w_ts = []
                for m in range(NFCH):
                    w_t = P1S.tile([128, DCH, 128], bf, tag="wstream")
                    f0 = CH_ORDER[m] * 128
                    nc.sync.dma_start(
                        out=w_t,
                        in_=wstT.ap()[:, f0:f0 + 128].rearrange(
                            "(c p) f -> p c f", p=128
                        ),
                    )
                    w_ts.append(w_t)

                # --- full norms (fp8 DoubleRow) + sum-of-squares ---
                # ones_mat reduction puts the sum in EVERY psum partition, so
                # sqrt/reciprocal write the [128,T] scale broadcast directly
                NQCH = QL // 128      # 12 qn chunks
                NCCH_T = (QL + KVL) // 128   # 16 total
                for t in range(NT):
                    psz_q = PZ1.tile([128, TS], f32, tag="pz", name="pszq")
                    psz_c = PZ1.tile([128, TS], f32, tag="pz", name="pszc")
                    for c in range(NCCH_T):
                        is_c = c >= NQCH
                        ps = PP1.tile([128, TS], f32, tag="pp")
                        for dp in range(DCH // 2):
                            nc.tensor.matmul(
                                ps,
                                lhsT=wn8_sb[:, 2 * dp:2 * dp + 2,
                                            c * 128:(c + 1) * 128],
                                rhs=x8_ts[t][:, 2 * dp:2 * dp + 2, :],
                                start=(dp == 0),
                                stop=(dp == DCH // 2 - 1),
                                perf_mode=DR,
                            )
                        sqv = SQS.tile([128, TS], bf, tag="sqs")
                        nc.scalar.activation(sqv, ps, AF.Square)
                        psz = psz_c if is_c else psz_q
                        nc.tensor.matmul(
                            psz,
                            lhsT=onesm_sb,
                            rhs=sqv,
                            start=(c == 0 or c == NQCH),
                            stop=(c == NQCH - 1 or c == NCCH_T - 1),
                        )
                        if c == NQCH - 1:
                            nc.scalar.activation(
                                sq_bc[:, ts(t, TS)], psz_q, AF.Sqrt,
                                bias=eps_sb,
                                scale=1.0 / (QL * NSCALE * NSCALE),
                            )
                        if c == NCCH_T - 1:
                            nc.scalar.activation(
                                sc_bc[:, ts(t, TS)], psz_c, AF.Sqrt,
                                bias=eps_sb,
                                scale=1.0 / (KVL * NSCALE * NSCALE),
                            )
                with nc.allow_low_precision(reason="rms scales bf16"):
                    nc.vector.reciprocal(sq_bc, sq_bc)
                    nc.vector.reciprocal(sc_bc, sc_bc)
                dr = DRS.tile([1, T], bf)
                nc.sync.dma_start(out=dr, in_=sc_bc[0:1, :])
                nc.sync.dma_start(
                    out=sc_colb,
                    in_=dr[:, :].rearrange("o (s p) -> (o p) s", p=128),
                )
                nc.vector.tensor_copy(sc_col, sc_colb)
                # rope tables pre-scaled by sq (in place)
                nc.vector.tensor_mul(ctab_sb, ctab_sb, sq_bc)
                nc.vector.tensor_mul(stab_sb, stab_sb, sq_bc)

                # --- main projection chunks ---
                for m in range(NFCH):
                    ch = CH_ORDER[m]
                    w_t = w_ts[m]
                    if ch == CH_V:
                        # V^T token-major: x slices are the stationary side
                        for s in range(SCH):
                            ps = PP1.tile([128, TS], f32, tag="pp")
                            for d in range(DCH):
                                nc.tensor.matmul(
                                    ps[:, 0:HD],
                                    lhsT=x_sb[d][:, s * 128:(s + 1) * 128],
                                    rhs=w_t[:, d, :],
                                    start=(d == 0),
                                    stop=(d == DCH - 1),
                                )
                            nc.vector.tensor_scalar_mul(
                                v_sb[:, s, :], ps[:, 0:HD], sc_col[:, s:s + 1]
                            )
                        continue
                    for t in range(NT):
                        ps = PP1.tile([128, TS], f32, tag="pp")
                        for d in range(DCH):
                            nc.tensor.matmul(
                                ps,
                                lhsT=w_t[:, d, :],
                                rhs=x_sb[d][:, ts(t, TS)],
                                start=(d == 0),
                                stop=(d == DCH - 1),
                            )
                        if ch == CH_K:
                            if SCORES_FP8:
                                nc.scalar.copy(keys_sb[:, 1, ts(t, TS)], ps)
                            else:
                                nc.vector.tensor_copy(k_sb[:, ts(t, TS)], ps)
                        elif ch == CH_KABS:
                            if SCORES_FP8:
                                tmp8 = SQS.tile([128, TS], bf, tag="c8")
                                nc.vector.tensor_mul(tmp8, ps,
                                                     sc_bc[:, ts(t, TS)])
                                nc.scalar.copy(keys_sb[:, 0, ts(t, TS)], tmp8)
                            else:
                                nc.vector.tensor_mul(kabs_sb[:, ts(t, TS)],
                                                     ps, sc_bc[:, ts(t, TS)])
                        elif ch < HPG:
                            h = ch
                            if SCORES_FP8:
                                tmp8 = SQS.tile([128, TS], bf, tag="c8")
                                nc.vector.tensor_mul(tmp8, ps,
                                                     sq_bc[:, ts(t, TS)])
                                nc.scalar.copy(q_sb[:, h, 0, ts(t, TS)], tmp8)
                            else:
                                nc.vector.tensor_mul(q_sb[:, h, ts(t, TS)],
                                                     ps, sq_bc[:, ts(t, TS)])
                        else:
                            h = ch - HPG
                            tsw = SQS.tile([128, TS], f32, tag="ropef")
                            nc.vector.stream_shuffle(tsw, ps, SWAP_MASK)
                            t1 = SQS.tile([128, TS], bf, tag="rope")
                            nc.vector.tensor_mul(t1, ps, ctab_sb[:, ts(t, TS)])
                            t2 = SQS.tile([128, TS], bf, tag="rope")
                            nc.vector.tensor_mul(t2, tsw, stab_sb[:, ts(t, TS)])
                            if SCORES_FP8:
                                tmp8 = SQS.tile([128, TS], bf, tag="c8")
                                nc.vector.tensor_add(tmp8, t1, t2)
                                nc.scalar.copy(q_sb[:, h, 1, ts(t, TS)], tmp8)
                            else:
                                nc.vector.tensor_add(qr_sb[:, h, ts(t, TS)],
                                                     t1, t2)

            # ---------------- phases 3+4: attention + out proj ---------
            if phases < 2:
                continue
            with tc.tile_pool(name="P3B", bufs=1) as P3B:
                ctx_sb = [
                    [
                        P3B.tile([128, TS], bf, tag=f"ctx{h}_{j}",
                                 name=f"ctx{h}_{j}")
                        for j in range(NT)
                    ]
                    for h in range(HPG)
                ]
                wout_sb = P3B.tile([128, HPG, T], bf)
                nc.gpsimd.dma_start(
                    out=wout_sb,
                    in_=woutT.ap().rearrange("(c p) e -> p c e", p=128),
                )
                masks_sb = P3B.tile([128, 4, TS], bf)
                nc.sync.dma_start(out=masks_sb, in_=masksd.ap())
                with (
                    tc.tile_pool(name="EP", bufs=8) as EP,
                    tc.tile_pool(name="TMPP", bufs=4) as TMPP,
                    tc.tile_pool(name="ZR", bufs=4) as ZR,
                    tc.tile_pool(name="OT", bufs=3) as OT,
                    tc.tile_pool(name="PSC", bufs=2, space="PSUM") as PSC,
                    tc.tile_pool(name="PCT", bufs=2, space="PSUM") as PCT,
                    tc.tile_pool(name="PZ3", bufs=2, space="PSUM") as PZ3,
                    tc.tile_pool(name="PO", bufs=2, space="PSUM") as PO,
                ):
                    # s-loop software pipeline: Z/ctx matmuls run LAG blocks
                    # behind the scores so the PE never waits on the
                    # PE->Act->DVE->PE e_t round-trip. The out-projection for
                    # query tile j-1 is emitted after attention tile j, so its
                    # ctx dependency is long satisfied when the PE reaches it.
                    LAG = 3

                    def emit_wout(jj):
                        for e in range(DCH):
                            po = PO.tile([128, TS], f32, tag="po")
                            for q in range(HPG):
                                nc.tensor.matmul(
                                    po,
                                    lhsT=wout_sb[:, q, e * 128:(e + 1) * 128],
                                    rhs=ctx_sb[q][jj],
                                    start=(q == 0),
                                    stop=(q == HPG - 1),
                                )
                            o_t = OT.tile([128, TS], f32, tag="ot")
                            nc.scalar.copy(o_t, po)
                            nc.gpsimd.dma_start(
                                out=outT.ap()[e * 128:(e + 1) * 128,
                                              jj * TS:(jj + 1) * TS],
                                in_=o_t,
                            )

                    for j in range(NT):
                        n_s = 4 * (j + 1)
                        for h in range(HPG):
                            psz = PZ3.tile([128, TS], f32, tag="pz3",
                                           name="psz")
                            pctx = PCT.tile([128, TS], f32, tag="pct",
                                            name="pctx")
                            e_list = {}
                            e8 = {}
                            for sv in range(n_s + LAG):
                                if sv < n_s:
                                    s_i = sv
                                    pss = PSC.tile([128, TS], f32, tag="psc")
                                    if SCORES_FP8:
                                        nc.tensor.matmul(
                                            pss,
                                            lhsT=keys_sb[:, :,
                                                         s_i * 128:(s_i + 1) * 128],
                                            rhs=q_sb[:, h, :, ts(j, TS)],
                                            start=True,
                                            stop=True,
                                            perf_mode=DR,
                                        )
                                    else:
                                        nc.tensor.matmul(
                                            pss,
                                            lhsT=kabs_sb[:,
                                                         s_i * 128:(s_i + 1) * 128],
                                            rhs=q_sb[:, h, ts(j, TS)],
                                            start=True,
                                            stop=False,
                                        )
                                        nc.tensor.matmul(
                                            pss,
                                            lhsT=k_sb[:, s_i * 128:(s_i + 1) * 128],
                                            rhs=qr_sb[:, h, ts(j, TS)],
                                            start=False,
                                            stop=True,
                                        )
                                    e_t = EP.tile([128, TS], bf, tag="e")
                                    nc.scalar.activation(e_t, pss, AF.Exp,
                                                         scale=SM_SCALE)
                                    if s_i >= 4 * j:
                                        nc.vector.tensor_mul(
                                            e_t, e_t, masks_sb[:, s_i - 4 * j, :]
                                        )
                                    e_list[s_i] = e_t
                                    if Z_FP8:
                                        if s_i % 2 == 0:
                                            e8[s_i // 2] = EP.tile(
                                                [128, 2, TS], fp8, tag="e8",
                                                name="e8t")
                                        nc.vector.tensor_copy(
                                            e8[s_i // 2][:, s_i % 2, :], e_t
                                        )
                                if sv >= LAG:
                                    s_o = sv - LAG
                                    et = e_list.pop(s_o)
                                    if Z_FP8:
                                        if s_o % 2 == 1:
                                            nc.tensor.matmul(
                                                psz,
                                                lhsT=ones8_sb,
                                                rhs=e8.pop(s_o // 2),
                                                start=(s_o == 1),
                                                stop=(s_o == n_s - 1),
                                                perf_mode=DR,
                                            )
                                    else:
                                        nc.tensor.matmul(
                                            psz,
                                            lhsT=onesm_sb,
                                            rhs=et,
                                            start=(s_o == 0),
                                            stop=(s_o == n_s - 1),
                                        )
                                    nc.tensor.matmul(
                                        pctx,
                                        lhsT=v_sb[:, s_o, :],
                                        rhs=et,
                                        start=(s_o == 0),
                                        stop=(s_o == n_s - 1),
                                    )
                            zcp = ZR.tile([128, TS], f32, tag="zcp")
                            nc.scalar.copy(zcp, psz)
                            zinv = ZR.tile([128, TS], f32, tag="zrow")
                            nc.vector.reciprocal(zinv, zcp)
                            nc.vector.tensor_mul(ctx_sb[h][j], pctx, zinv)
                        if fuse_out and j > 0:
                            emit_wout(j - 1)
                    if fuse_out:
                        emit_wout(NT - 1)
                    else:
                        for jj in range(NT):
                            emit_wout(jj)

    nc.compile()
    return nc


def _get_program():
    if "nc" not in _CACHE:
        _CACHE["nc"] = _build_program()
    return _CACHE["nc"]


def _host_prep(inputs):
    """Fold weights on the host and build the 8 per-core input maps."""
    x = np.asarray(inputs["x"], np.float32)
    Wdq = np.asarray(inputs["Wdq"], np.float32)
    qw = np.asarray(inputs["q_norm_w"], np.float32)
    Wuq = np.asarray(inputs["Wuq"], np.float32) * qw[None, :]
    Wqr = np.asarray(inputs["Wqr"], np.float32) * qw[None, :]
    Wdkv = np.asarray(inputs["Wdkv"], np.float32)
    kvw = np.asarray(inputs["kv_norm_w"], np.float32)
    Wuk = np.asarray(inputs["Wuk"], np.float32) * kvw[None, :]
    Wuv = np.asarray(inputs["Wuv"], np.float32) * kvw[None, :]
    Wkr = np.asarray(inputs["Wkr"], np.float32)
    Wout = np.asarray(inputs["Wout"], np.float32)

    # folded projections
    Wq_eff = Wuq @ Wdq          # [H*HD, D]
    Wqr_eff = Wqr @ Wdq
    Wk_eff = Wuk @ Wdkv         # [HD, D]
    Wv_eff = Wuv @ Wdkv         # [KVH*HD, D]

    inv = 1.0 / (10000.0 ** (np.arange(0, HD, 2, dtype=np.float32) / HD))
    f = np.arange(T, dtype=np.float32)[None, :] * inv[:, None]   # [64, T]
    cosT, sinT = np.cos(f), np.sin(f)
    Ctab = np.repeat(cosT, 2, axis=0)                            # [128, T]
    Stab = np.repeat(sinT, 2, axis=0)
    Stab[0::2, :] *= -1.0                                        # pair-swap sign

    fH = np.arange(KVH, dtype=np.float32)[None, :] * inv[:, None]  # [64, KVH]
    cosH, sinH = np.cos(fH), np.sin(fH)

    def bft(a):
        return np.ascontiguousarray(a).astype(BF16)

    def f8t(a):
        return np.ascontiguousarray(a).astype(FP8)

    ctab_b = bft(Ctab)
    stab_b = bft(Stab)
    masks_np = np.ones((128, 4, TS), np.float32)
    for r in range(4):
        for p in range(128):
            masks_np[p, r, :p + 128 * r] = 0.0
    masks_b = bft(masks_np)

    in_maps = []
    for b in range(B):
        x_T = bft(x[b].T)
        x8 = f8t(x[b].T)
        for g in range(G):
            # fold K-rope (fixed rotation per kv-head index) into Wkr
            Wkr_g = Wkr[g * HD:(g + 1) * HD, :]
            we, wo = Wkr_g[0::2, :], Wkr_g[1::2, :]
            c_g, s_g = cosH[:, g][:, None], sinH[:, g][:, None]
            Wkr_eff = np.empty_like(Wkr_g)
            Wkr_eff[0::2, :] = we * c_g - wo * s_g
            Wkr_eff[1::2, :] = we * s_g + wo * c_g

            wst = np.concatenate(
                [
                    Wq_eff[g * HPG * HD:(g + 1) * HPG * HD],
                    Wqr_eff[g * HPG * HD:(g + 1) * HPG * HD],
                    Wk_eff,
                    Wkr_eff,
                    Wv_eff[g * HD:(g + 1) * HD],
                ],
                axis=0,
            )  # [NF, D]
            wn = np.concatenate([Wdq, Wdkv], axis=0) * NSCALE  # [QL+KVL, D]

            in_maps.append(
                dict(
                    x_T=x_T,
                    x8=x8,
                    wstT=bft(wst.T),
                    wn8T=f8t(wn.T),
                    woutT=bft(Wout[:, g * HPG * HD:(g + 1) * HPG * HD].T),
                    ctab=ctab_b,
                    stab=stab_b,
                    masks=masks_b,
                )
            )
    return in_maps


def kernel(**inputs):
    global LAST_RESULTS
    from concourse import bass_utils

    nc = _get_program()
    in_maps = _host_prep(inputs)
    res = bass_utils.run_bass_kernel_spmd(
        nc, in_maps, core_ids=list(range(NCORES))
    )
    LAST_RESULTS = res
    out = np.zeros((B, T, D), np.float32)
    for i, r in enumerate(res.results):
        out[i // G] += r["out_T"].T
    return out
